# revision 4
# baseline (speedup 1.0000x reference)
"""GAT layer (nn_GATLayer) on 8 Trainium2 NeuronCores via Bass/Tile.

Reference computation (N=8192, F=512, D=64):
    z = features @ W                      # [N, D]
    s = z @ a_self; t = z @ a_neigh       # [N, 1]
    e[i,j] = leakyrelu(s[i] + t[j], 0.2)
    attention = softmax(e + mask(A), axis=1)   # mask: -1e12 where A<=0
    h = attention @ z                     # [N, D]

Sharding: row-shard the N x N attention across 8 cores (1024 rows each).
Each core computes z for its own feature rows, all-gathers z (augmented
with a ones column so the softmax denominator falls out of the same
matmul), then streams its [8192 x 1024] transposed block of A while
computing  E[j, i] = exp(leakyrelu(s_i + t_j)) * A[i, j]  and
accumulating  H_aug[d, i] = sum_j z_aug[j, d] * E[j, i]  on the PE.
Row 64 of H_aug is the softmax denominator; the epilogue transposes
H_aug back, multiplies by its reciprocal, and writes h rows.

Scores are laid out transposed ([j partitions, i free]) so the softmax
reduction and the PV contraction are both over j on the PE partition
axis; no max-subtraction is needed (scores are O(10), exp is safe), and
masking multiplies by A in {0,1} after exp instead of adding -1e12
before it.
"""

import sys

sys.path.insert(0, "/opt/trn_rl_repo")

import numpy as np

N, F, D = 8192, 512, 64
NCORES = 8
R = N // NCORES          # rows per core (1024)
JC = N // 128            # j-chunks (64)
DP = D + 1               # z augmented with ones column (65)
ALPHA = 0.2

_CACHE = {}


def _build_program():
    import concourse.bacc as bacc
    import concourse.tile as tile
    from concourse import mybir
    from concourse.masks import make_identity

    f32 = mybir.dt.float32
    f32r = mybir.dt.float32r
    Alu = mybir.AluOpType
    Act = mybir.ActivationFunctionType

    nc = bacc.Bacc("TRN2", target_bir_lowering=False, debug=False, num_devices=NCORES)

    feat_t = nc.dram_tensor("feat_t", [F, R], f32, kind="ExternalInput")
    a_t = nc.dram_tensor("a_t", [N, R], f32, kind="ExternalInput")
    w_in = nc.dram_tensor("w", [F, D], f32, kind="ExternalInput")
    a_self = nc.dram_tensor("a_self", [D, 1], f32, kind="ExternalInput")
    a_neigh = nc.dram_tensor("a_neigh", [1, D], f32, kind="ExternalInput")
    h_out = nc.dram_tensor("h", [R, D], f32, kind="ExternalOutput")

    with tile.TileContext(nc) as tc:
        with (
            tc.tile_pool(name="const", bufs=1) as cst,
            tc.tile_pool(name="dram", bufs=1, space="DRAM") as dram,
            tc.tile_pool(name="ps_main", bufs=1, space="PSUM") as ps_main,
        ):
            # ---- constants / prologue inputs ----
            ft = cst.tile([128, 4 * R], f32)        # features^T, F on partitions
            for c in range(4):
                nc.sync.dma_start(out=ft[:, c * R:(c + 1) * R],
                                  in_=feat_t[c * 128:(c + 1) * 128, :])
            w_sb = cst.tile([128, 4 * D], f32)
            for c in range(4):
                nc.sync.dma_start(out=w_sb[:, c * D:(c + 1) * D],
                                  in_=w_in[c * 128:(c + 1) * 128, :])
            asf = cst.tile([D, 1], f32)
            nc.sync.dma_start(out=asf[:], in_=a_self[:])
            anp = cst.tile([1, DP], f32)            # a_neigh padded with 0
            nc.vector.memset(anp[:], 0.0)
            nc.sync.dma_start(out=anp[0:1, 0:D], in_=a_neigh[:])
            ones1 = cst.tile([1, 128], f32)
            nc.vector.memset(ones1[:], 1.0)

            hp = ps_main.tile([DP, R], f32)          # H_aug accumulator

            with tc.tile_pool(name="ps_pro", bufs=2, space="PSUM") as ps_pro:
                # ---- z_aug_local = [features_local @ W | 1]  ([R, DP]) ----
                zaug_local = dram.tile([R, DP], f32)
                for ib in range(R // 128):
                    psz = ps_pro.tile([128, D], f32, tag="pro")
                    for c in range(4):
                        nc.tensor.matmul(
                            psz[:],
                            ft[:, c * R + ib * 128: c * R + (ib + 1) * 128],
                            w_sb[:, c * D:(c + 1) * D],
                            start=(c == 0), stop=(c == 3),
                        )
                    zb = cst.tile([128, DP], f32, tag="zb")
                    nc.vector.tensor_copy(zb[:, 0:D], psz[:])
                    nc.vector.memset(zb[:, D:DP], 1.0)
                    nc.sync.dma_start(
                        out=zaug_local[ib * 128:(ib + 1) * 128, :], in_=zb[:])

                # ---- all-gather z_aug ----
                zaug_full = dram.tile([N, DP], f32, addr_space="Shared")
                nc.gpsimd.collective_compute(
                    "AllGather", Alu.bypass,
                    replica_groups=[list(range(NCORES))],
                    ins=[zaug_local.opt()], outs=[zaug_full.opt()],
                )
                zf = cst.tile([128, JC, DP], f32)    # z_aug, j-chunked
                nc.sync.dma_start(
                    out=zf[:],
                    in_=zaug_full[:].rearrange("(c p) d -> p c d", p=128))
                zf_r = cst.tile([128, JC, DP], f32r)  # f32r copy for PE stationary
                nc.vector.tensor_copy(zf_r[:], zf[:])

                # ---- z_local^T (D on partitions) for the s row ----
                pzt = ps_pro.tile([D, R], f32, tag="pro")
                for c in range(4):
                    for hh in range(2):
                        nc.tensor.matmul(
                            pzt[:, hh * 512:(hh + 1) * 512],
                            w_sb[:, c * D:(c + 1) * D],
                            ft[:, c * R + hh * 512: c * R + (hh + 1) * 512],
                            start=(c == 0), stop=(c == 3),
                        )
                zt_sb = cst.tile([D, R], f32)
                nc.vector.tensor_copy(zt_sb[:], pzt[:])

                # ---- s row and its broadcast across partitions ----
                pss = ps_pro.tile([1, R], f32, tag="pro")
                for hh in range(2):
                    nc.tensor.matmul(
                        pss[:, hh * 512:(hh + 1) * 512],
                        asf[:],
                        zt_sb[:, hh * 512:(hh + 1) * 512],
                        start=True, stop=True,
                    )
                s_sb = cst.tile([1, R], f32)
                nc.vector.tensor_copy(s_sb[:], pss[:])
                psb = ps_pro.tile([128, R], f32, tag="pro")
                for hh in range(2):
                    nc.tensor.matmul(
                        psb[:, hh * 512:(hh + 1) * 512],
                        ones1[:],
                        s_sb[0:1, hh * 512:(hh + 1) * 512],
                        start=True, stop=True,
                    )
                s_bcast = cst.tile([128, R], f32)
                nc.vector.tensor_copy(s_bcast[:], psb[:])

                # ---- a_neigh broadcast and t per j-chunk ----
                pan = ps_pro.tile([128, DP], f32, tag="pro")
                nc.tensor.matmul(pan[:], ones1[:], anp[:], start=True, stop=True)
                anb = cst.tile([128, DP], f32)
                nc.vector.tensor_copy(anb[:], pan[:])

                t_sb = cst.tile([128, JC], f32)
                for jc in range(JC):
                    scr = cst.tile([128, DP], f32, tag="tscr")
                    nc.vector.tensor_tensor(scr[:], zf[:, jc], anb[:], Alu.mult)
                    nc.vector.tensor_reduce(
                        t_sb[:, jc:jc + 1], scr[:], mybir.AxisListType.X, Alu.add)

            # ---- main loop over j-chunks ----
            with (
                tc.tile_pool(name="a_pool", bufs=6) as a_pool,
                tc.tile_pool(name="work", bufs=3) as work,
            ):
                for jc in range(JC):
                    at = a_pool.tile([128, R], f32, tag="at")
                    nc.sync.dma_start(
                        out=at[:], in_=a_t[jc * 128:(jc + 1) * 128, :])

                    u = work.tile([128, R], f32, tag="u")
                    nc.scalar.activation(
                        u[:], s_bcast[:], Act.Prelu,
                        bias=t_sb[:, jc:jc + 1], scale=1.0, alpha=ALPHA)
                    e = work.tile([128, R], f32, tag="e")
                    nc.scalar.activation(e[:], u[:], Act.Exp)
                    ea = work.tile([128, R], f32r, tag="ea")
                    nc.vector.tensor_tensor(ea[:], e[:], at[:], Alu.mult)

                    for hh in range(2):
                        nc.tensor.matmul(
                            hp[:, hh * 512:(hh + 1) * 512],
                            zf_r[:, jc],
                            ea[:, hh * 512:(hh + 1) * 512],
                            start=(jc == 0), stop=(jc == JC - 1),
                        )

            # ---- epilogue: transpose H_aug, normalize, store ----
            with (
                tc.tile_pool(name="ps_epi", bufs=2, space="PSUM") as ps_epi,
                tc.tile_pool(name="epi", bufs=2) as epi,
            ):
                h_sb = cst.tile([DP, R], f32)
                nc.vector.tensor_copy(h_sb[:], hp[:])
                ident = cst.tile([DP, DP], f32)
                make_identity(nc, ident[:])
                for b in range(R // 128):
                    trp = ps_epi.tile([128, DP], f32, tag="trp")
                    nc.tensor.transpose(
                        trp[:], h_sb[:, b * 128:(b + 1) * 128], ident[:])
                    rec = epi.tile([128, 1], f32, tag="rec")
                    nc.vector.reciprocal(rec[:], trp[:, D:DP])
                    hb = epi.tile([128, D], f32, tag="hb")
                    nc.vector.tensor_scalar_mul(hb[:], trp[:, 0:D], rec[:, 0:1])
                    nc.sync.dma_start(
                        out=h_out[b * 128:(b + 1) * 128, :], in_=hb[:])

    nc.compile()
    return nc


def _get_program():
    if "nc" not in _CACHE:
        _CACHE["nc"] = _build_program()
    return _CACHE["nc"]


def kernel(features, A, W, a_self, a_neigh):
    from concourse.bass_utils import run_bass_kernel_spmd

    nc = _get_program()

    features = np.asarray(features, dtype=np.float32)
    A = np.asarray(A)
    W = np.ascontiguousarray(np.asarray(W, dtype=np.float32))
    a_self_c = np.ascontiguousarray(np.asarray(a_self, dtype=np.float32).reshape(D, 1))
    a_neigh_c = np.ascontiguousarray(np.asarray(a_neigh, dtype=np.float32).reshape(1, D))

    in_maps = []
    for k in range(NCORES):
        rows = slice(k * R, (k + 1) * R)
        in_maps.append({
            "feat_t": np.ascontiguousarray(features[rows, :].T),
            "a_t": A[rows, :].T.astype(np.float32),
            "w": W,
            "a_self": a_self_c,
            "a_neigh": a_neigh_c,
        })

    res = run_bass_kernel_spmd(nc, in_maps, list(range(NCORES)))
    h = np.concatenate([res.results[k]["h"] for k in range(NCORES)], axis=0)
    return h.astype(np.float32)
